# revision 5
# baseline (speedup 1.0000x reference)
"""Trainium2 Bass kernel for nn_EncoderStack (single-head attention + 2-layer GELU FFN).

Sharding: 8 cores = 4 batch elements x 2 sequence halves. Each core computes
K,V for its batch element's full 2048-token sequence (redundantly with its
pair core) and Q/attention/FFN for its own 1024-token half. No collectives.

Layout strategy (per core):
  - Activations enter feature-major (x^T, D on partitions) so every GEMM
    chains without transposes:
      Q^T,K^T feature-major = MM(lhsT=W, rhs=x^T)
      V row-major           = MM(lhsT=x^T, rhs=Wv)
      scores^T              = MM(lhsT=K^T, rhs=Q^T)   (k on partitions)
      attn row-major        = MM(lhsT=exp_scores^T, rhs=V)
      softmax sums          = MM(lhsT=exp_scores^T, rhs=ones)
  - Softmax skips max-subtraction (|scores/8| <= ~11, exp fits fp32 easily).
  - LayerNorms run row-major via bn_stats/bn_aggr; the only transpose in the
    whole kernel is h -> h^T (PE transpose) feeding the FFN.
  - Matmul operands are bf16 (fp32 PSUM accumulation); residual/LN math fp32.
"""

import sys

sys.path.insert(0, "/opt/trn_rl_repo")

import numpy as np
import ml_dtypes

import concourse.bass as bass
import concourse.tile as tile
from concourse import bacc, mybir
from concourse.bass_utils import run_bass_kernel_spmd
from concourse.masks import make_identity

P = 128
D = 1024
S = 2048          # full sequence per batch element
SQ = 1024         # this core's query rows
DS = D // P       # 8 d-subtiles
KS = S // P       # 16 key blocks
QB = SQ // P      # 8 query row-blocks
QTILE = 256       # attention q-tile
NQT = SQ // QTILE # 4
EPS = 1e-5
SCALE = 0.125     # 1/sqrt(d_k) = 1/8

F32 = mybir.dt.float32
BF16 = mybir.dt.bfloat16
Act = mybir.ActivationFunctionType
Alu = mybir.AluOpType

_NC_CACHE = {}


def _ln_rowmajor(nc, pool, t, out, g_b, b_b, eps_t):
    """Row-major layernorm: out = (t - mean)/sqrt(var+eps) * g + b.

    t: [128, 1024] fp32 sbuf tile (rows on partitions). g_b/b_b: [128,1024]
    broadcast tiles. out may have any dtype.
    """
    stats = pool.tile([P, 2, 6], F32, tag="ln_stats")
    nc.vector.bn_stats(out=stats[:, 0, :], in_=t[:, 0:512])
    nc.vector.bn_stats(out=stats[:, 1, :], in_=t[:, 512:1024])
    mv = pool.tile([P, 2], F32, tag="ln_mv")
    nc.vector.bn_aggr(out=mv[:], in_=stats[:])
    std = pool.tile([P, 1], F32, tag="ln_std")
    nc.scalar.activation(out=std[:], in_=mv[:, 1:2], func=Act.Sqrt, bias=eps_t[:])
    rstd = pool.tile([P, 1], F32, tag="ln_rstd")
    nc.vector.reciprocal(out=rstd[:], in_=std[:])
    negmean = pool.tile([P, 1], F32, tag="ln_negmean")
    nc.vector.tensor_scalar_mul(negmean[:], mv[:, 0:1], -1.0)
    v = pool.tile([P, D], F32, tag="ln_v")
    # v = (t - mean) * g
    nc.vector.scalar_tensor_tensor(
        out=v[:], in0=t[:], scalar=negmean[:], in1=g_b[:], op0=Alu.add, op1=Alu.mult
    )
    # out = v * rstd + b
    nc.vector.scalar_tensor_tensor(
        out=out[:], in0=v[:], scalar=rstd[:], in1=b_b[:], op0=Alu.mult, op1=Alu.add
    )


def _build_nc():
    nc = bacc.Bacc(None)

    xt = nc.dram_tensor("xt", [P, DS, S], BF16, kind="ExternalInput")
    xq = nc.dram_tensor("xq", [P, QB, D], F32, kind="ExternalInput")
    wq = nc.dram_tensor("wq", [P, DS, D], BF16, kind="ExternalInput")
    wk = nc.dram_tensor("wk", [P, DS, D], BF16, kind="ExternalInput")
    wv = nc.dram_tensor("wv", [P, DS, D], BF16, kind="ExternalInput")
    w1 = nc.dram_tensor("w1", [P, DS, D], BF16, kind="ExternalInput")
    w2 = nc.dram_tensor("w2", [P, DS, D], BF16, kind="ExternalInput")
    bqc = nc.dram_tensor("bqc", [P, DS], F32, kind="ExternalInput")
    bkc = nc.dram_tensor("bkc", [P, DS], F32, kind="ExternalInput")
    c1c = nc.dram_tensor("c1c", [P, DS], F32, kind="ExternalInput")
    # broadcast rows
    bvr = nc.dram_tensor("bvr", [1, D], F32, kind="ExternalInput")
    c2r = nc.dram_tensor("c2r", [1, D], F32, kind="ExternalInput")
    gatr = nc.dram_tensor("gatr", [1, D], F32, kind="ExternalInput")
    batr = nc.dram_tensor("batr", [1, D], F32, kind="ExternalInput")
    glnr = nc.dram_tensor("glnr", [1, D], F32, kind="ExternalInput")
    blnr = nc.dram_tensor("blnr", [1, D], F32, kind="ExternalInput")
    out = nc.dram_tensor("out", [P, QB, D], F32, kind="ExternalOutput")

    with tile.TileContext(nc) as tc:
        with (
            tc.tile_pool(name="singles", bufs=1) as singles,
            tc.tile_pool(name="kv", bufs=1) as kv,
            tc.tile_pool(name="dram", bufs=1, space="DRAM") as dram,
        ):
            # ---- constants ----
            bvb = singles.tile([P, D], F32)
            c2b = singles.tile([P, D], F32)
            gatb = singles.tile([P, D], F32)
            batb = singles.tile([P, D], F32)
            glnb = singles.tile([P, D], F32)
            blnb = singles.tile([P, D], F32)
            for t_, r_ in ((bvb, bvr), (c2b, c2r), (gatb, gatr), (batb, batr),
                           (glnb, glnr), (blnb, blnr)):
                nc.sync.dma_start(out=t_[:], in_=r_[:].to_broadcast((P, D)))
            bqt = singles.tile([P, DS], F32)
            bkt = singles.tile([P, DS], F32)
            c1t = singles.tile([P, DS], F32)
            nc.sync.dma_start(out=bqt[:], in_=bqc[:])
            nc.sync.dma_start(out=bkt[:], in_=bkc[:])
            nc.sync.dma_start(out=c1t[:], in_=c1c[:])
            eps_t = singles.tile([P, 1], F32)
            nc.vector.memset(eps_t[:], EPS)
            ones_t = singles.tile([P, 1], BF16)
            nc.vector.memset(ones_t[:], 1.0)
            ident = singles.tile([P, P], BF16)
            make_identity(nc, ident[:])

            # ---- persistent SBUF tensors ----
            KT = kv.tile([P, DS, S], BF16)    # K^T feature-major
            V = kv.tile([P, KS, D], BF16)     # V row-major
            # DRAM spills
            QT_d = dram.tile([P, DS, SQ], BF16)   # Q^T feature-major
            H_d = dram.tile([P, QB, D], F32)      # h (post 2nd LN) row-major
            HT_d = dram.tile([P, DS, SQ], BF16)   # h^T feature-major

            # ================= Phase A: projections =================
            with (
                tc.tile_pool(name="wA", bufs=1) as wA,
                tc.tile_pool(name="xa", bufs=3) as xa,
                tc.tile_pool(name="eva", bufs=3) as eva,
                tc.tile_pool(name="psA", bufs=2, space="PSUM") as psA,
            ):
                wq_t = wA.tile([P, DS, D], BF16)
                wk_t = wA.tile([P, DS, D], BF16)
                wv_t = wA.tile([P, DS, D], BF16)
                nc.sync.dma_start(out=wq_t[:], in_=wq[:])
                nc.sync.dma_start(out=wk_t[:], in_=wk[:])
                nc.sync.dma_start(out=wv_t[:], in_=wv[:])

                for sc in range(4):  # 512-wide s-chunks over full sequence
                    xt_t = xa.tile([P, DS, 512], BF16, tag="xt")
                    nc.sync.dma_start(out=xt_t[:], in_=xt[:, :, sc * 512:(sc + 1) * 512])

                    # K^T[:, db, sc] (feature-major)
                    for db in range(DS):
                        ps = psA.tile([P, 512], F32, tag="psk")
                        for ks in range(DS):
                            nc.tensor.matmul(
                                ps[:],
                                wk_t[:, ks, db * P:(db + 1) * P],
                                xt_t[:, ks, :],
                                start=(ks == 0), stop=(ks == DS - 1),
                            )
                        nc.scalar.activation(
                            out=KT[:, db, sc * 512:(sc + 1) * 512], in_=ps[:],
                            func=Act.Identity, bias=bkt[:, db:db + 1],
                        )

                    # V rows (row-major), V[s,:] for s in this chunk
                    for rb in range(4):
                        for dn in range(2):
                            ps = psA.tile([P, 512], F32, tag="psv")
                            for ks in range(DS):
                                nc.tensor.matmul(
                                    ps[:],
                                    xt_t[:, ks, rb * P:(rb + 1) * P],
                                    wv_t[:, ks, dn * 512:(dn + 1) * 512],
                                    start=(ks == 0), stop=(ks == DS - 1),
                                )
                            nc.vector.scalar_tensor_tensor(
                                out=V[:, sc * 4 + rb, dn * 512:(dn + 1) * 512],
                                in0=ps[:], scalar=1.0,
                                in1=bvb[:, dn * 512:(dn + 1) * 512],
                                op0=Alu.mult, op1=Alu.add,
                            )

                    # Q^T (own half = first two chunks only)
                    if sc < 2:
                        for db in range(DS):
                            ps = psA.tile([P, 512], F32, tag="psq")
                            for ks in range(DS):
                                nc.tensor.matmul(
                                    ps[:],
                                    wq_t[:, ks, db * P:(db + 1) * P],
                                    xt_t[:, ks, :],
                                    start=(ks == 0), stop=(ks == DS - 1),
                                )
                            qev = eva.tile([P, 512], BF16, tag="qev")
                            nc.scalar.activation(
                                out=qev[:], in_=ps[:],
                                func=Act.Identity, bias=bqt[:, db:db + 1],
                            )
                            nc.sync.dma_start(
                                out=QT_d[:, db, sc * 512:(sc + 1) * 512], in_=qev[:]
                            )

            # ================= Phase B: attention + LN1/LN2 =================
            with (
                tc.tile_pool(name="qtp", bufs=2) as qtp,
                tc.tile_pool(name="expp", bufs=2) as expp,
                tc.tile_pool(name="attp", bufs=2) as attp,
                tc.tile_pool(name="xqp", bufs=2) as xqp,
                tc.tile_pool(name="lnB", bufs=3) as lnB,
                tc.tile_pool(name="hbp", bufs=2) as hbp,
                tc.tile_pool(name="htev", bufs=3) as htev,
                tc.tile_pool(name="psS", bufs=2, space="PSUM") as psS,
                tc.tile_pool(name="psM", bufs=2, space="PSUM") as psM,
                tc.tile_pool(name="psA2", bufs=2, space="PSUM") as psA2,
                tc.tile_pool(name="psT", bufs=2, space="PSUM") as psT,
            ):
                for qt in range(NQT):
                    qt_t = qtp.tile([P, DS, QTILE], BF16, tag="qt")
                    nc.sync.dma_start(
                        out=qt_t[:], in_=QT_d[:, :, qt * QTILE:(qt + 1) * QTILE]
                    )
                    expT = expp.tile([P, KS, QTILE], BF16, tag="expT")
                    for ks in range(KS):
                        ps = psS.tile([P, QTILE], F32, tag="pss")
                        for ds in range(DS):
                            nc.tensor.matmul(
                                ps[:],
                                KT[:, ds, ks * P:(ks + 1) * P],
                                qt_t[:, ds, :],
                                start=(ds == 0), stop=(ds == DS - 1),
                            )
                        nc.scalar.activation(
                            out=expT[:, ks, :], in_=ps[:], func=Act.Exp, scale=SCALE
                        )

                    for qb in range(2):
                        rbq = qt * 2 + qb  # global 128-row block index
                        qsl = slice(qb * P, (qb + 1) * P)

                        pssum = psM.tile([P, 1], F32, tag="pssum")
                        for ks in range(KS):
                            nc.tensor.matmul(
                                pssum[:], expT[:, ks, qsl], ones_t[:],
                                start=(ks == 0), stop=(ks == KS - 1),
                            )
                        recip = lnB.tile([P, 1], F32, tag="recip")
                        nc.vector.reciprocal(out=recip[:], in_=pssum[:])

                        xq_t = xqp.tile([P, D], F32, tag="xq")
                        nc.sync.dma_start(out=xq_t[:], in_=xq[:, rbq, :])

                        t = attp.tile([P, D], F32, tag="att_t")
                        for dn in range(2):
                            psa = psA2.tile([P, 512], F32, tag="psa")
                            for ks in range(KS):
                                nc.tensor.matmul(
                                    psa[:],
                                    expT[:, ks, qsl],
                                    V[:, ks, dn * 512:(dn + 1) * 512],
                                    start=(ks == 0), stop=(ks == KS - 1),
                                )
                            # t = attn/sum + x  (fused scale + residual)
                            nc.vector.scalar_tensor_tensor(
                                out=t[:, dn * 512:(dn + 1) * 512], in0=psa[:],
                                scalar=recip[:], in1=xq_t[:, dn * 512:(dn + 1) * 512],
                                op0=Alu.mult, op1=Alu.add,
                            )

                        at = attp.tile([P, D], F32, tag="at_t")
                        _ln_rowmajor(nc, lnB, t, at, gatb, batb, eps_t)
                        t2 = attp.tile([P, D], F32, tag="t2_t")
                        nc.vector.tensor_add(out=t2[:], in0=at[:], in1=xq_t[:])
                        h = attp.tile([P, D], F32, tag="h_t")
                        _ln_rowmajor(nc, lnB, t2, h, glnb, blnb, eps_t)
                        nc.sync.dma_start(out=H_d[:, rbq, :], in_=h[:])

                        hb = hbp.tile([P, D], BF16, tag="hb")
                        nc.vector.tensor_copy(out=hb[:], in_=h[:])
                        for ds in range(DS):
                            pst = psT.tile([P, P], BF16, tag="pst")
                            nc.tensor.transpose(
                                pst[:], hb[:, ds * P:(ds + 1) * P], ident[:]
                            )
                            hte = htev.tile([P, P], BF16, tag="hte")
                            nc.scalar.copy(out=hte[:], in_=pst[:])
                            nc.sync.dma_start(
                                out=HT_d[:, ds, rbq * P:(rbq + 1) * P], in_=hte[:]
                            )

            # ================= Phase C: FFN + final LN =================
            with (
                tc.tile_pool(name="wC", bufs=1) as wC,
                tc.tile_pool(name="htp", bufs=2) as htp,
                tc.tile_pool(name="g1p", bufs=2) as g1p,
                tc.tile_pool(name="g2p", bufs=2) as g2p,
                tc.tile_pool(name="hrp", bufs=2) as hrp,
                tc.tile_pool(name="lnC", bufs=3) as lnC,
                tc.tile_pool(name="outp", bufs=2) as outp,
                tc.tile_pool(name="psC", bufs=4, space="PSUM") as psC,
            ):
                w1_t = wC.tile([P, DS, D], BF16)
                w2_t = wC.tile([P, DS, D], BF16)
                nc.sync.dma_start(out=w1_t[:], in_=w1[:])
                nc.sync.dma_start(out=w2_t[:], in_=w2[:])

                for qt2 in range(2):  # 512-wide q-tiles
                    ht_t = htp.tile([P, DS, 512], BF16, tag="ht")
                    nc.sync.dma_start(
                        out=ht_t[:], in_=HT_d[:, :, qt2 * 512:(qt2 + 1) * 512]
                    )
                    g1T = g1p.tile([P, DS, 512], BF16, tag="g1T")
                    for eb in range(DS):
                        ps = psC.tile([P, 512], F32, tag="psc1")
                        for ds in range(DS):
                            nc.tensor.matmul(
                                ps[:],
                                w1_t[:, ds, eb * P:(eb + 1) * P],
                                ht_t[:, ds, :],
                                start=(ds == 0), stop=(ds == DS - 1),
                            )
                        nc.scalar.activation(
                            out=g1T[:, eb, :], in_=ps[:], func=Act.Gelu,
                            bias=c1t[:, eb:eb + 1],
                        )

                    for qb in range(4):
                        rbq = qt2 * 4 + qb
                        qsl = slice(qb * P, (qb + 1) * P)
                        g2 = g2p.tile([P, D], F32, tag="g2")
                        for dn in range(2):
                            ps = psC.tile([P, 512], F32, tag="psc2")
                            for es in range(DS):
                                nc.tensor.matmul(
                                    ps[:],
                                    g1T[:, es, qsl],
                                    w2_t[:, es, dn * 512:(dn + 1) * 512],
                                    start=(es == 0), stop=(es == DS - 1),
                                )
                            nc.vector.scalar_tensor_tensor(
                                out=g2[:, dn * 512:(dn + 1) * 512], in0=ps[:],
                                scalar=1.0, in1=c2b[:, dn * 512:(dn + 1) * 512],
                                op0=Alu.mult, op1=Alu.add,
                            )
                        g2g = g2p.tile([P, D], F32, tag="g2g")
                        nc.scalar.activation(out=g2g[:], in_=g2[:], func=Act.Gelu)

                        h_t = hrp.tile([P, D], F32, tag="hres")
                        nc.sync.dma_start(out=h_t[:], in_=H_d[:, rbq, :])
                        nc.vector.tensor_add(out=g2g[:], in0=g2g[:], in1=h_t[:])
                        o = outp.tile([P, D], F32, tag="o")
                        _ln_rowmajor(nc, lnC, g2g, o, glnb, blnb, eps_t)
                        nc.sync.dma_start(out=out[:, rbq, :], in_=o[:])
    nc.compile()
    return nc


def get_nc():
    if "nc" not in _NC_CACHE:
        _NC_CACHE["nc"] = _build_nc()
    return _NC_CACHE["nc"]


def _part_major(a, dtype):
    """[D0*P? -> no: (ds*P+p, n) array] -> [P, ds, n] partition-major."""
    r, n = a.shape
    ds = r // P
    return np.ascontiguousarray(
        a.reshape(ds, P, n).transpose(1, 0, 2)
    ).astype(dtype)


def _prep_in_maps(x, Wq, bq, Wk, bk, Wv, bv, g_at, b_at, g_ln, b_ln, W1, c1, W2, c2):
    bf = ml_dtypes.bfloat16
    shared = {
        "wq": _part_major(Wq, bf), "wk": _part_major(Wk, bf),
        "wv": _part_major(Wv, bf), "w1": _part_major(W1, bf),
        "w2": _part_major(W2, bf),
        "bqc": np.ascontiguousarray(bq.reshape(DS, P).T).astype(np.float32),
        "bkc": np.ascontiguousarray(bk.reshape(DS, P).T).astype(np.float32),
        "c1c": np.ascontiguousarray(c1.reshape(DS, P).T).astype(np.float32),
        "bvr": bv.reshape(1, D).astype(np.float32),
        "c2r": c2.reshape(1, D).astype(np.float32),
        "gatr": g_at.reshape(1, D).astype(np.float32),
        "batr": b_at.reshape(1, D).astype(np.float32),
        "glnr": g_ln.reshape(1, D).astype(np.float32),
        "blnr": b_ln.reshape(1, D).astype(np.float32),
    }
    in_maps = []
    for core in range(8):
        b, half = core // 2, core % 2
        own = x[b, half * SQ:(half + 1) * SQ]          # [1024, 1024]
        other = x[b, (1 - half) * SQ:(2 - half) * SQ]  # [1024, 1024]
        # x^T with own half first: [D, 2048]
        xtb = np.concatenate([own.T, other.T], axis=1)
        in_maps.append({
            **shared,
            "xt": _part_major(xtb, bf),
            "xq": _part_major(own, np.float32),
        })
    return in_maps


def _assemble(results):
    out = np.empty((4, S, D), np.float32)
    for core, r in enumerate(results):
        b, half = core // 2, core % 2
        o = r["out"]  # [P, QB, D]
        out[b, half * SQ:(half + 1) * SQ] = (
            o.transpose(1, 0, 2).reshape(SQ, D)
        )
    return out


def run(trace=False, **inputs):
    nc = get_nc()
    in_maps = _prep_in_maps(**{k: np.asarray(v) for k, v in inputs.items()})
    res = run_bass_kernel_spmd(nc, in_maps, list(range(8)), trace=trace)
    return _assemble(res.results), res


def kernel(**inputs):
    out, _ = run(trace=False, **inputs)
    return out


if __name__ == "__main__":
    import reference as R
    inputs = R.setup_inputs()
    inputs = {k: np.asarray(v) for k, v in inputs.items()}
    out = kernel(**inputs)
    import jax.numpy as jnp
    exp = np.asarray(R.reference(**{k: jnp.asarray(v) for k, v in inputs.items()}))
    err = np.abs(out - exp)
    print("max abs err:", err.max(), "scale:", np.abs(exp).max())
    print("rel (scale):", err.max() / np.abs(exp).max())
